# revision 29
# baseline (speedup 1.0000x reference)
"""DeepseekV3 MoE layer on 8 Trainium2 NeuronCores.

Strategy (expert-parallel, per sharding hint):
- Each core owns 2 of the 16 routed experts. The host routes tokens to cores
  by top-k index lists (the all-to-all dispatch, done as input sharding): each
  core receives its experts' gathered tokens pre-transposed to [H, C] fp16,
  plus the per-token combine weights for its experts (host-side routing).
- The device runs the SwiGLU expert MLP in fp16 (fp32 PSUM accumulation),
  scales expert outputs by the combine weights, and scatter-adds them into a
  partial-output buffer in DRAM.
- The shared expert is sharded along its intermediate dim (128 of 1024 per
  core); its partial output initializes the partial-output buffer.
- One ReduceScatter sums the partials across cores; its result is copied to
  the output. The host reassembles the 8 slices and casts to fp32.

All inputs are host-packed partition-major so every DMA moves ~128 fat
descriptors (the DMA rings drain at descriptor-count parity, not bytes).
"""

import os
import sys
import types

sys.path.insert(0, "/opt/trn_rl_repo")

# antenv.axon_hooks shim so trace=True works under axon (profiling only).
if "antenv.axon_hooks" not in sys.modules:
    _hook_holder = [None]
    _hooks_mod = types.ModuleType("antenv.axon_hooks")
    _hooks_mod.set_axon_ntff_profile_hook = lambda h: _hook_holder.__setitem__(0, h)
    _hooks_mod.get_axon_ntff_profile_hook = lambda: _hook_holder[0]
    sys.modules["antenv.axon_hooks"] = _hooks_mod
    try:
        from trn_agent_boot.trn_boot import _ntff_profile_via_ctypes

        _hook_holder[0] = _ntff_profile_via_ctypes("/opt/axon/libaxon_pjrt.so")
    except Exception:
        pass

import numpy as np

import concourse.bass as bass
import concourse.mybir as mybir
from concourse import bacc
from concourse.tile import TileContext, add_dep_helper
from concourse.bass_utils import run_bass_kernel_spmd

N_CORES = 8
T, H, E, I = 2048, 1024, 16, 512
TOPK = 4
SIC = 128  # shared-expert intermediate slice per core (1024 / 8)
EPC = 2  # experts per core
OOB = 1 << 20
NOLD = bool(int(os.environ.get("KERNEL_NOLD", "1")))  # skip dup weight loads
NCH = int(os.environ.get("KERNEL_NCH", "2"))  # reduce-scatter token chunks

F16 = mybir.dt.float16
F32 = mybir.dt.float32
I32 = mybir.dt.int32
AF = mybir.ActivationFunctionType

_nc_cache = {}
last_exec_time_ns = None


def _build(C_use, C_pad, edges, scat_tiles, touch_sets):
    NCC = C_pad // 128
    NTI = T // 128
    SS = 2 * SIC  # 256
    ROWS = T // N_CORES  # 256 output rows per core
    nc = bacc.Bacc(trn_type="TRN2", target_bir_lowering=False, num_devices=N_CORES)

    # ---- I/O (all packed partition-major on the host) ----
    xTq = nc.dram_tensor("xTq", [4, 128, H // 128, T // 4], F16, kind="ExternalInput")
    xgT16 = nc.dram_tensor("xgT16", [EPC, 128, H // 128, C_pad], F16, kind="ExternalInput")
    wg16 = nc.dram_tensor("wg16", [EPC, 128, H // 128, I], F16, kind="ExternalInput")
    wu16 = nc.dram_tensor("wu16", [EPC, 128, H // 128, I], F16, kind="ExternalInput")
    wd16 = nc.dram_tensor("wd16", [EPC, 128, I // 128, H], F16, kind="ExternalInput")
    sgsu16 = nc.dram_tensor("sgsu16", [128, H // 128, SS], F16, kind="ExternalInput")
    sd16 = nc.dram_tensor("sd16", [SIC, H], F16, kind="ExternalInput")
    sidx = nc.dram_tensor("sidx", [128, EPC * NCC], I32, kind="ExternalInput")
    wgt = nc.dram_tensor("wgt", [128, EPC * NCC, EPC], F32, kind="ExternalInput")
    ident = nc.dram_tensor("ident", [128, 128], F16, kind="ExternalInput")

    y_acc = nc.dram_tensor("y_acc", [T, H], F16)
    rs_b = nc.dram_tensor("rs_b", [ROWS, H], F16)
    y_out = nc.dram_tensor("y_out", [ROWS, H], F16, kind="ExternalOutput")

    with TileContext(nc) as tc:
        with (
            tc.tile_pool(name="res", bufs=1) as res,
            tc.tile_pool(name="sc", bufs=3) as scp,
            tc.tile_pool(name="yg", bufs=4) as ygp,
            tc.tile_pool(name="ps", bufs=2, space="PSUM") as ps,
        ):
            # ---- resident tiles ----
            # quarter-major so each xT quarter lands as one 8KB run/partition
            xT_sb = res.tile([128, 4, H // 128, T // 4], F16, tag="xT")
            xgT_sb = res.tile([128, EPC, H // 128, C_pad], F16, tag="xgT")
            wg_sb = res.tile([128, EPC, H // 128, I], F16, tag="wg")
            wu_sb = res.tile([128, EPC, H // 128, I], F16, tag="wu")
            wd_sb = res.tile([128, EPC, I // 128, H], F16, tag="wd")
            sgsu_sb = res.tile([128, H // 128, SS], F16, tag="sgsu")
            sd_sb = res.tile([128, H], F16, tag="sd")
            sidx_sb = res.tile([128, EPC * NCC], I32, tag="sidx")
            wG_sb = res.tile([128, EPC * NCC, EPC], F32, tag="wG")
            id_sb = res.tile([128, 128], F16, tag="ident")
            p_sb = res.tile([128, EPC, I // 128, C_pad], F16, tag="p")
            sp_sb = res.tile([128, NTI, SIC], F16, tag="sp")
            spT_sb = res.tile([128, NTI, 128], F16, tag="spT")

            # ---- preload (sgsu + xT q0 gate the first matmuls) ----
            sgsu_wr = nc.scalar.dma_start(sgsu_sb[:], sgsu16.ap())
            nc.scalar.dma_start(id_sb[:], ident[:])
            nc.scalar.dma_start(sd_sb[:], sd16.ap())
            nc.scalar.dma_start(sidx_sb[:], sidx.ap())
            nc.scalar.dma_start(wG_sb[:], wgt.ap())

            TC = T // 4
            xt_wr = []
            for q in range(4):
                xt_wr.append(nc.sync.dma_start(xT_sb[:, q], xTq.ap()[q]))
            # gate all bulk traffic on the two loads that unblock compute
            # (the tile scheduler reorders DMAs, so every one needs the dep)
            bulk = []
            for e in range(EPC):
                bulk.append(nc.sync.dma_start(xgT_sb[:, e], xgT16.ap()[e]))
            for e in range(EPC):
                bulk.append(nc.gpsimd.dma_start(wg_sb[:, e], wg16.ap()[e]))
                bulk.append(nc.gpsimd.dma_start(wu_sb[:, e], wu16.ap()[e]))
            for e in range(EPC):
                bulk.append(nc.gpsimd.dma_start(wd_sb[:, e], wd16.ap()[e]))
            for b in bulk:
                add_dep_helper(b.ins, xt_wr[0].ins, reason="priority: xT q0 first")
                add_dep_helper(b.ins, sgsu_wr.ins, reason="priority: sgsu first")



            # ---- shared expert, fused per token tile: gate/up, silu*up,
            # transpose, down, y_acc init ----
            dense_wr = []
            for ti in range(NTI):
                psu = ps.tile([128, SS], F32, tag="A")
                for ho in range(H // 128):
                    nc.tensor.matmul(
                        psu[:],
                        lhsT=xT_sb[:, ti // 4, ho, (ti % 4) * 128:(ti % 4 + 1) * 128],
                        rhs=sgsu_sb[:, ho, :],
                        start=(ho == 0),
                        stop=(ho == H // 128 - 1),
                    )
                sg_t = scp.tile([128, SIC], F16, tag="sg_t")
                nc.scalar.activation(sg_t[:], psu[:, 0:SIC], AF.Silu)
                nc.vector.tensor_tensor(
                    out=sp_sb[:, ti, :], in0=sg_t[:], in1=psu[:, SIC:SS],
                    op=mybir.AluOpType.mult,
                )
                tps = ps.tile([128, 128], F16, tag="B")
                nc.tensor.transpose(tps[:], sp_sb[:, ti, :], id_sb[:])
                nc.vector.tensor_copy(spT_sb[:, ti, :], tps[:])
                ysh = ps.tile([128, H], F32, tag="Y")
                for hf in range(2):
                    mm = nc.tensor.matmul(
                        ysh[:, hf * 512:(hf + 1) * 512],
                        lhsT=spT_sb[:, ti, :],
                        rhs=sd_sb[:, hf * 512:(hf + 1) * 512],
                        start=True,
                        stop=True,
                    )
                    if NOLD and hf == 1:
                        mm.ins.ldweights = False
                ys = ygp.tile([128, H], F16, tag="ys", bufs=4)
                nc.scalar.activation(ys[:], ysh[:], AF.Copy)
                # alternate rings: each ring drains 2KB-descriptor writes
                # at ~87GB/s, so two rings halve the dense-init drain
                q = nc.sync if ti % 2 == 0 else nc.scalar
                wr = q.dma_start(out=y_acc[ti * 128:(ti + 1) * 128, :], in_=ys[:])
                dense_wr.append(wr)

            # ---- routed experts: per 128-token chunk, fully fused:
            # gate/up (token-stationary: one xgT tile load feeds both 512-wide
            # streams), silu*up, transpose back to I-major, down, scale,
            # scatter. Scatters start ~45us in and overlap all later compute.
            scat_insts = {}

            def emit_gu_cc(e, cc):
                a = cc * 128
                pgT = ps.tile([128, I], F32, tag="A")
                puT = ps.tile([128, I], F32, tag="B")
                for ho in range(H // 128):
                    nc.tensor.matmul(
                        pgT[:],
                        lhsT=xgT_sb[:, e, ho, a:a + 128],
                        rhs=wg_sb[:, e, ho, :],
                        start=(ho == 0),
                        stop=(ho == H // 128 - 1),
                    )
                    nc.tensor.matmul(
                        puT[:],
                        lhsT=xgT_sb[:, e, ho, a:a + 128],
                        rhs=wu_sb[:, e, ho, :],
                        start=(ho == 0),
                        stop=(ho == H // 128 - 1),
                    )
                sg2 = scp.tile([128, I], F16, tag="sg2")
                nc.scalar.activation(sg2[:], pgT[:], AF.Silu)
                sp2 = scp.tile([128, I], F16, tag="sp2")
                nc.vector.tensor_tensor(
                    out=sp2[:], in0=sg2[:], in1=puT[:],
                    op=mybir.AluOpType.mult,
                )
                for it in range(I // 128):
                    tps2 = ps.tile([128, 128], F16, tag="B")
                    nc.tensor.transpose(
                        tps2[:], sp2[:, it * 128:(it + 1) * 128], id_sb[:])
                    nc.vector.tensor_copy(p_sb[:, e, it, a:a + 128], tps2[:])

            def emit_down(e, cc):
                j = e * NCC + cc
                py = ps.tile([128, H], F32, tag="Y")
                for it in range(I // 128):
                    for hf in range(2):
                        mm = nc.tensor.matmul(
                            py[:, hf * 512:(hf + 1) * 512],
                            lhsT=p_sb[:, e, it, cc * 128:(cc + 1) * 128],
                            rhs=wd_sb[:, e, it, hf * 512:(hf + 1) * 512],
                            start=(it == 0),
                            stop=(it == I // 128 - 1),
                        )
                        if NOLD and hf == 1:
                            mm.ins.ldweights = False
                yg = ygp.tile([128, H], F16, tag="ygtile", bufs=6)
                nc.vector.tensor_scalar_mul(yg[:], py[:], wG_sb[:, j, e:e + 1])

                sc = nc.gpsimd.indirect_dma_start(
                    out=y_acc[:],
                    out_offset=bass.IndirectOffsetOnAxis(
                        ap=sidx_sb[:, j:j + 1], axis=0),
                    in_=yg[:],
                    in_offset=None,
                    bounds_check=T - 1,
                    oob_is_err=False,
                    compute_op=mybir.AluOpType.add,
                )
                # order RMW scatters after the dense init of the tiles they
                # touch, and serialize only colliding scatter pairs
                for t in scat_tiles[(e, cc)]:
                    add_dep_helper(sc.ins, dense_wr[t].ins, reason="scatter after dense init")
                for (oe, occ) in edges.get((e, cc), ()):
                    if (oe, occ) in scat_insts:
                        add_dep_helper(sc.ins, scat_insts[(oe, occ)].ins,
                                       reason="serialize colliding scatters")
                scat_insts[(e, cc)] = sc

            # ---- token-chunked ReduceScatter, triggered progressively ----
            rs_insts = []
            CH = T // NCH
            RPC = ROWS // NCH

            def emit_rs(k):
                cc_inst = nc.gpsimd.collective_compute(
                    "ReduceScatter",
                    mybir.AluOpType.add,
                    replica_groups=[list(range(N_CORES))],
                    ins=[y_acc.ap()[k * CH:(k + 1) * CH, :].opt()],
                    outs=[rs_b.ap()[k * RPC:(k + 1) * RPC, :].opt()],
                )
                for key in touch_sets[k]:
                    add_dep_helper(cc_inst.ins, scat_insts[key].ins,
                                   reason="rs after scatters")
                for t in range(k * (NTI // NCH), (k + 1) * (NTI // NCH)):
                    add_dep_helper(cc_inst.ins, dense_wr[t].ins,
                                   reason="rs after dense init")
                rs_insts.append(cc_inst)

            def try_emit_rs():
                while len(rs_insts) < NCH and all(
                        key in scat_insts for key in touch_sets[len(rs_insts)]):
                    emit_rs(len(rs_insts))

            for cc in range(NCC):
                for e in range(EPC):
                    emit_gu_cc(e, cc)
                    emit_down(e, cc)
                try_emit_rs()
            try_emit_rs()
            assert len(rs_insts) == NCH, f"only {len(rs_insts)} RS chunks emitted"

            for k in range(NCH):
                cp = nc.sync.dma_start(
                    y_out.ap()[k * RPC:(k + 1) * RPC, :],
                    rs_b.ap()[k * RPC:(k + 1) * RPC, :])
                add_dep_helper(cp.ins, rs_insts[k].ins, reason="copy rs out")

    nc.compile()
    return nc


def _get_nc(C_use, C_pad, edges, scat_tiles, touch_sets):
    key = (C_use, C_pad, NOLD, NCH,
           tuple(sorted((k, tuple(v)) for k, v in edges.items())),
           tuple(sorted(scat_tiles.items())), touch_sets)
    if key not in _nc_cache:
        _nc_cache[key] = _build(C_use, C_pad, edges, scat_tiles, touch_sets)
    return _nc_cache[key]


def kernel(hidden_states, gate_w, expert_gate, expert_up, expert_down,
           shared_gate, shared_up, shared_down):
    global last_exec_time_ns
    B, S, Hh = hidden_states.shape
    x = np.asarray(hidden_states, np.float32).reshape(-1, Hh)

    # ---- host-side routing: top-k expert choice + combine weights ----
    gw = np.asarray(gate_w, np.float32)
    logits = x @ gw.T
    scores = 1.0 / (1.0 + np.exp(-logits))
    # top-4 per token; stable sort matches jax.lax.top_k tie semantics
    order = np.argsort(-scores, axis=1, kind="stable")[:, :TOPK]
    topk_w = np.take_along_axis(scores, order, axis=1)
    topk_w = topk_w / (topk_w.sum(-1, keepdims=True) + 1e-20)
    w2 = np.zeros((T, E), np.float32)
    np.put_along_axis(w2, order, topk_w, axis=1)
    sel = w2 > 0
    counts = sel.sum(0)
    C_use = int(max(64, -(-int(counts.max()) // 64) * 64))
    C_use = min(C_use, T)
    C_pad = -(-C_use // 128) * 128
    NCC = C_pad // 128

    gidx_all = np.zeros((E, C_pad), np.int32)
    sidx_all = np.full((E, C_pad), OOB, np.int32)
    for e in range(E):
        lst = np.nonzero(sel[:, e])[0].astype(np.int32)
        gidx_all[e, :len(lst)] = lst
        sidx_all[e, :len(lst)] = lst

    # ---- cast / pack per-core inputs (the all-to-all token dispatch),
    # partition-major so each DMA is 128 fat descriptors ----
    x16 = x.astype(np.float16)
    # [4 quarters][128 part (h%128)][8 (h//128)][512 tokens]
    xTq = np.ascontiguousarray(
        x16.T.reshape(H // 128, 128, 4, T // 4).transpose(2, 1, 0, 3))
    eg = np.asarray(expert_gate, np.float32).astype(np.float16)
    eu = np.asarray(expert_up, np.float32).astype(np.float16)
    ed = np.asarray(expert_down, np.float32).astype(np.float16)
    sg = np.asarray(shared_gate, np.float32).astype(np.float16)
    su = np.asarray(shared_up, np.float32).astype(np.float16)
    sd = np.asarray(shared_down, np.float32).astype(np.float16)
    identity = np.eye(128, dtype=np.float16)

    def pack_w(w):  # [H, I] -> [128, H//128, I]
        return np.ascontiguousarray(w.reshape(-1, 128, w.shape[-1]).transpose(1, 0, 2))

    in_maps = []
    for c in range(N_CORES):
        ex = [EPC * c + k for k in range(EPC)]
        xgT = np.stack([
            np.ascontiguousarray(
                x16[gidx_all[e]].T.reshape(H // 128, 128, C_pad).transpose(1, 0, 2))
            for e in ex
        ])
        sgsu = np.concatenate(
            [sg[:, c * SIC:(c + 1) * SIC], su[:, c * SIC:(c + 1) * SIC]], axis=1)
        # combine weights in gathered layout: [128][e*NCC+cc][local e]
        wgt = np.zeros((128, EPC * NCC, EPC), np.float32)
        for k, e in enumerate(ex):
            wgt[:, k * NCC:(k + 1) * NCC, k] = \
                w2[gidx_all[e].reshape(NCC, 128), e].T
        in_maps.append({
            "xTq": xTq,
            "xgT16": xgT,
            "wg16": np.stack([pack_w(eg[e]) for e in ex]),
            "wu16": np.stack([pack_w(eu[e]) for e in ex]),
            "wd16": np.stack([pack_w(ed[e]) for e in ex]),
            "sgsu16": pack_w(sgsu),
            "sd16": np.ascontiguousarray(sd[c * SIC:(c + 1) * SIC, :]),
            "sidx": np.ascontiguousarray(
                sidx_all[ex].reshape(EPC * NCC, 128).T),
            "wgt": wgt,
            "ident": identity,
        })

    # scatter collision edges (union across cores so the SPMD program is
    # identical everywhere), dense-init tiles each scatter touches, and
    # which scatters touch each RS token chunk
    CH = T // NCH
    edge_set = set()
    sctiles = {(k, i): set() for k in range(EPC) for i in range(NCC)}
    touch = [set() for _ in range(NCH)]
    rng = {}
    for c in range(N_CORES):
        for k, e in enumerate([EPC * c, EPC * c + 1]):
            for i in range(NCC):
                r = sidx_all[e, i * 128:(i + 1) * 128]
                r = r[r < OOB]
                if len(r):
                    lo, hi = int(r.min()), int(r.max())
                    rng[(c, k, i)] = (lo, hi)
                    for t in range(lo // 128, hi // 128 + 1):
                        sctiles[(k, i)].add(t)
                    for h in range(lo // CH, hi // CH + 1):
                        touch[h].add((k, i))
        for i in range(NCC):
            for j in range(NCC):
                a = rng.get((c, 0, i))
                b = rng.get((c, 1, j))
                if a and b and a[0] <= b[1] and b[0] <= a[1]:
                    # emission order: (0,cc), (1,cc) per cc ascending
                    if i <= j:
                        edge_set.add(((1, j), (0, i)))
                    else:
                        edge_set.add(((0, i), (1, j)))
    edges = {}
    for later, earlier in edge_set:
        edges.setdefault(later, []).append(earlier)
    edges = {k: tuple(sorted(v)) for k, v in edges.items()}
    scat_tiles = {key: tuple(sorted(v)) for key, v in sctiles.items()}
    touch_sets = tuple(frozenset(s) for s in touch)

    nc = _get_nc(C_use, C_pad, edges, scat_tiles, touch_sets)
    trace = bool(int(os.environ.get("KERNEL_TRACE", "0")))
    res = run_bass_kernel_spmd(
        nc, in_maps, core_ids=list(range(N_CORES)), trace=trace
    )
    last_exec_time_ns = res.exec_time_ns

    # reassemble: RS chunk k gives core c tokens [k*CH + c*RPC, +RPC)
    RPC = T // N_CORES // NCH
    out = np.empty((T, Hh), np.float32)
    for c in range(N_CORES):
        yo = res.results[c]["y_out"]
        for k in range(NCH):
            out[k * CH + c * RPC:k * CH + (c + 1) * RPC] = yo[k * RPC:(k + 1) * RPC]
    return out.reshape(B, S, Hh).astype(np.float32)


# revision 33
# speedup vs baseline: 1.0823x; 1.0823x over previous
"""DeepseekV3 MoE layer on 8 Trainium2 NeuronCores.

Strategy (expert-parallel, per sharding hint):
- Each core owns 2 of the 16 routed experts. The host routes tokens to cores
  by top-k index lists (the all-to-all dispatch, done as input sharding): each
  core receives its experts' gathered tokens pre-transposed to [H, C] fp16,
  plus the per-token combine weights for its experts (host-side routing).
- The device runs the SwiGLU expert MLP in fp16 (fp32 PSUM accumulation),
  scales expert outputs by the combine weights, and scatter-adds them into a
  partial-output buffer in DRAM.
- The shared expert is sharded along its intermediate dim (128 of 1024 per
  core); its partial output initializes the partial-output buffer.
- One ReduceScatter sums the partials across cores; its result is copied to
  the output. The host reassembles the 8 slices and casts to fp32.

All inputs are host-packed partition-major so every DMA moves ~128 fat
descriptors (the DMA rings drain at descriptor-count parity, not bytes).
"""

import os
import sys
import types

sys.path.insert(0, "/opt/trn_rl_repo")

# antenv.axon_hooks shim so trace=True works under axon (profiling only).
if "antenv.axon_hooks" not in sys.modules:
    _hook_holder = [None]
    _hooks_mod = types.ModuleType("antenv.axon_hooks")
    _hooks_mod.set_axon_ntff_profile_hook = lambda h: _hook_holder.__setitem__(0, h)
    _hooks_mod.get_axon_ntff_profile_hook = lambda: _hook_holder[0]
    sys.modules["antenv.axon_hooks"] = _hooks_mod
    try:
        from trn_agent_boot.trn_boot import _ntff_profile_via_ctypes

        _hook_holder[0] = _ntff_profile_via_ctypes("/opt/axon/libaxon_pjrt.so")
    except Exception:
        pass

import numpy as np

import concourse.bass as bass
import concourse.mybir as mybir
from concourse import bacc
from concourse.tile import TileContext, add_dep_helper
from concourse.bass_utils import run_bass_kernel_spmd

N_CORES = 8
T, H, E, I = 2048, 1024, 16, 512
TOPK = 4
SIC = 128  # shared-expert intermediate slice per core (1024 / 8)
EPC = 2  # experts per core
OOB = 1 << 20
NOLD = bool(int(os.environ.get("KERNEL_NOLD", "1")))  # skip dup weight loads
NCH = int(os.environ.get("KERNEL_NCH", "1"))  # reduce-scatter token chunks

F16 = mybir.dt.float16
F32 = mybir.dt.float32
I32 = mybir.dt.int32
AF = mybir.ActivationFunctionType

_nc_cache = {}
last_exec_time_ns = None


def _build(C_use, C_pad, edges, scat_tiles, touch_sets):
    NCC = C_pad // 128
    NTI = T // 128
    SS = 2 * SIC  # 256
    ROWS = T // N_CORES  # 256 output rows per core
    nc = bacc.Bacc(trn_type="TRN2", target_bir_lowering=False, num_devices=N_CORES)

    # ---- I/O (all packed partition-major on the host) ----
    xTq = nc.dram_tensor("xTq", [4, 128, H // 128, T // 4], F16, kind="ExternalInput")
    xgT16 = nc.dram_tensor("xgT16", [EPC, 128, H // 128, C_pad], F16, kind="ExternalInput")
    wg16 = nc.dram_tensor("wg16", [EPC, 128, H // 128, I], F16, kind="ExternalInput")
    wu16 = nc.dram_tensor("wu16", [EPC, 128, H // 128, I], F16, kind="ExternalInput")
    wd16 = nc.dram_tensor("wd16", [EPC, 128, I // 128, H], F16, kind="ExternalInput")
    sgsu16 = nc.dram_tensor("sgsu16", [128, H // 128, SS], F16, kind="ExternalInput")
    sd16 = nc.dram_tensor("sd16", [SIC, H], F16, kind="ExternalInput")
    sidx = nc.dram_tensor("sidx", [128, EPC * NCC], I32, kind="ExternalInput")
    wgt = nc.dram_tensor("wgt", [128, EPC * NCC, EPC], F32, kind="ExternalInput")
    ident = nc.dram_tensor("ident", [128, 128], F16, kind="ExternalInput")

    y_acc = nc.dram_tensor("y_acc", [T, H], F16)
    rs_b = nc.dram_tensor("rs_b", [ROWS, H], F16)
    y_out = nc.dram_tensor("y_out", [ROWS, H], F16, kind="ExternalOutput")

    with TileContext(nc) as tc:
        with (
            tc.tile_pool(name="res", bufs=1) as res,
            tc.tile_pool(name="sc", bufs=3) as scp,
            tc.tile_pool(name="yg", bufs=4) as ygp,
            tc.tile_pool(name="ps", bufs=2, space="PSUM") as ps,
        ):
            # ---- resident tiles ----
            # quarter-major so each xT quarter lands as one 8KB run/partition
            xT_sb = res.tile([128, 4, H // 128, T // 4], F16, tag="xT")
            xgT_sb = res.tile([128, EPC, H // 128, C_pad], F16, tag="xgT")
            wg_sb = res.tile([128, EPC, H // 128, I], F16, tag="wg")
            wu_sb = res.tile([128, EPC, H // 128, I], F16, tag="wu")
            wd_sb = res.tile([128, EPC, I // 128, H], F16, tag="wd")
            sgsu_sb = res.tile([128, H // 128, SS], F16, tag="sgsu")
            sd_sb = res.tile([128, H], F16, tag="sd")
            sidx_sb = res.tile([128, EPC * NCC], I32, tag="sidx")
            wG_sb = res.tile([128, EPC * NCC, EPC], F32, tag="wG")
            id_sb = res.tile([128, 128], F16, tag="ident")
            p_sb = res.tile([128, EPC, I // 128, C_pad], F16, tag="p")
            spT_sb = res.tile([128, T], F16, tag="spT")

            # ---- preload (sgsu + xT q0 gate the first matmuls) ----
            sgsu_wr = nc.scalar.dma_start(sgsu_sb[:], sgsu16.ap())
            nc.scalar.dma_start(id_sb[:], ident[:])
            nc.scalar.dma_start(sd_sb[:], sd16.ap())
            nc.scalar.dma_start(sidx_sb[:], sidx.ap())
            nc.scalar.dma_start(wG_sb[:], wgt.ap())

            TC = T // 4
            xt_wr = []
            for q in range(4):
                xt_wr.append(nc.sync.dma_start(xT_sb[:, q], xTq.ap()[q]))
            # gate all bulk traffic on the two loads that unblock compute
            # (the tile scheduler reorders DMAs, so every one needs the dep);
            # xgT also waits for the last xT quarter so the shared block's
            # xT stream is never starved
            bulk = []
            xg_wr = []
            for e in range(EPC):
                xg_wr.append(nc.sync.dma_start(xgT_sb[:, e], xgT16.ap()[e]))
            for e in range(EPC):
                bulk.append(nc.gpsimd.dma_start(wg_sb[:, e], wg16.ap()[e]))
                bulk.append(nc.gpsimd.dma_start(wu_sb[:, e], wu16.ap()[e]))
            for e in range(EPC):
                bulk.append(nc.gpsimd.dma_start(wd_sb[:, e], wd16.ap()[e]))
            for b in bulk + xg_wr:
                add_dep_helper(b.ins, xt_wr[0].ins, reason="priority: xT q0 first")
                add_dep_helper(b.ins, sgsu_wr.ins, reason="priority: sgsu first")
            for b in xg_wr:
                add_dep_helper(b.ins, xt_wr[3].ins, reason="xT quarters first")



            # ---- shared expert, per xT quarter, weight-stationary so the
            # intermediate comes out already transposed (no PE transposes):
            # sgT/suT = (x @ sg|su)^T accumulate [128 si, 512 tok] ----
            dense_wr = []
            for q in range(4):
                sgp = ps.tile([128, TC], F32, tag="A")
                for ho in range(H // 128):
                    nc.tensor.matmul(
                        sgp[:],
                        lhsT=sgsu_sb[:, ho, 0:SIC],
                        rhs=xT_sb[:, q, ho, :],
                        start=(ho == 0),
                        stop=(ho == H // 128 - 1),
                    )
                sgq = scp.tile([128, TC], F16, tag="sgq")
                nc.scalar.activation(sgq[:], sgp[:], AF.Silu)
                sup = ps.tile([128, TC], F32, tag="B")
                for ho in range(H // 128):
                    nc.tensor.matmul(
                        sup[:],
                        lhsT=sgsu_sb[:, ho, SIC:SS],
                        rhs=xT_sb[:, q, ho, :],
                        start=(ho == 0),
                        stop=(ho == H // 128 - 1),
                    )
                nc.vector.tensor_tensor(
                    out=spT_sb[:, q * TC:(q + 1) * TC], in0=sgq[:], in1=sup[:],
                    op=mybir.AluOpType.mult,
                )
                for ti in range(4 * q, 4 * q + 4):
                    ysh = ps.tile([128, H], F32, tag="Y")
                    for hf in range(2):
                        nc.tensor.matmul(
                            ysh[:, hf * 512:(hf + 1) * 512],
                            lhsT=spT_sb[:, ti * 128:(ti + 1) * 128],
                            rhs=sd_sb[:, hf * 512:(hf + 1) * 512],
                            start=True,
                            stop=True,
                        )
                    ys = ygp.tile([128, H], F16, tag="ys", bufs=4)
                    nc.scalar.activation(ys[:], ysh[:], AF.Copy)
                    # alternate rings: each drains 2KB-descriptor writes at
                    # ~87GB/s, so two rings halve the dense-init drain
                    qr = nc.sync if ti % 2 == 0 else nc.scalar
                    wr = qr.dma_start(out=y_acc[ti * 128:(ti + 1) * 128, :], in_=ys[:])
                    dense_wr.append(wr)

            # ---- routed experts: per 128-token chunk, fully fused:
            # gate/up (token-stationary: one xgT tile load feeds both 512-wide
            # streams), silu*up, transpose back to I-major, down, scale,
            # scatter. Scatters start ~45us in and overlap all later compute.
            scat_insts = {}

            def emit_gu_cc(e, cc):
                a = cc * 128
                pgT = ps.tile([128, I], F32, tag="A")
                puT = ps.tile([128, I], F32, tag="B")
                for ho in range(H // 128):
                    nc.tensor.matmul(
                        pgT[:],
                        lhsT=xgT_sb[:, e, ho, a:a + 128],
                        rhs=wg_sb[:, e, ho, :],
                        start=(ho == 0),
                        stop=(ho == H // 128 - 1),
                    )
                    nc.tensor.matmul(
                        puT[:],
                        lhsT=xgT_sb[:, e, ho, a:a + 128],
                        rhs=wu_sb[:, e, ho, :],
                        start=(ho == 0),
                        stop=(ho == H // 128 - 1),
                    )
                sg2 = scp.tile([128, I], F16, tag="sg2")
                nc.scalar.activation(sg2[:], pgT[:], AF.Silu)
                sp2 = scp.tile([128, I], F16, tag="sp2")
                nc.vector.tensor_tensor(
                    out=sp2[:], in0=sg2[:], in1=puT[:],
                    op=mybir.AluOpType.mult,
                )
                for it in range(I // 128):
                    tps2 = ps.tile([128, 128], F16, tag="B")
                    nc.tensor.transpose(
                        tps2[:], sp2[:, it * 128:(it + 1) * 128], id_sb[:])
                    nc.vector.tensor_copy(p_sb[:, e, it, a:a + 128], tps2[:])

            def emit_down(e, cc):
                j = e * NCC + cc
                py = ps.tile([128, H], F32, tag="Y")
                for it in range(I // 128):
                    for hf in range(2):
                        mm = nc.tensor.matmul(
                            py[:, hf * 512:(hf + 1) * 512],
                            lhsT=p_sb[:, e, it, cc * 128:(cc + 1) * 128],
                            rhs=wd_sb[:, e, it, hf * 512:(hf + 1) * 512],
                            start=(it == 0),
                            stop=(it == I // 128 - 1),
                        )
                        if NOLD and hf == 1:
                            mm.ins.ldweights = False
                yg = ygp.tile([128, H], F16, tag="ygtile", bufs=6)
                nc.vector.tensor_scalar_mul(yg[:], py[:], wG_sb[:, j, e:e + 1])

                sc = nc.gpsimd.indirect_dma_start(
                    out=y_acc[:],
                    out_offset=bass.IndirectOffsetOnAxis(
                        ap=sidx_sb[:, j:j + 1], axis=0),
                    in_=yg[:],
                    in_offset=None,
                    bounds_check=T - 1,
                    oob_is_err=False,
                    compute_op=mybir.AluOpType.add,
                )
                # order RMW scatters after the dense init of the tiles they
                # touch, and serialize only colliding scatter pairs
                for t in scat_tiles[(e, cc)]:
                    add_dep_helper(sc.ins, dense_wr[t].ins, reason="scatter after dense init")
                for (oe, occ) in edges.get((e, cc), ()):
                    if (oe, occ) in scat_insts:
                        add_dep_helper(sc.ins, scat_insts[(oe, occ)].ins,
                                       reason="serialize colliding scatters")
                scat_insts[(e, cc)] = sc

            # ---- token-chunked ReduceScatter, triggered progressively ----
            rs_insts = []
            CH = T // NCH
            RPC = ROWS // NCH

            def emit_rs(k):
                cc_inst = nc.gpsimd.collective_compute(
                    "ReduceScatter",
                    mybir.AluOpType.add,
                    replica_groups=[list(range(N_CORES))],
                    ins=[y_acc.ap()[k * CH:(k + 1) * CH, :].opt()],
                    outs=[rs_b.ap()[k * RPC:(k + 1) * RPC, :].opt()],
                )
                for key in touch_sets[k]:
                    add_dep_helper(cc_inst.ins, scat_insts[key].ins,
                                   reason="rs after scatters")
                for t in range(k * (NTI // NCH), (k + 1) * (NTI // NCH)):
                    add_dep_helper(cc_inst.ins, dense_wr[t].ins,
                                   reason="rs after dense init")
                rs_insts.append(cc_inst)

            def try_emit_rs():
                while len(rs_insts) < NCH and all(
                        key in scat_insts for key in touch_sets[len(rs_insts)]):
                    emit_rs(len(rs_insts))

            for cc in range(NCC):
                for e in range(EPC):
                    emit_gu_cc(e, cc)
                    emit_down(e, cc)
                try_emit_rs()
            try_emit_rs()
            assert len(rs_insts) == NCH, f"only {len(rs_insts)} RS chunks emitted"

            for k in range(NCH):
                cp = nc.sync.dma_start(
                    y_out.ap()[k * RPC:(k + 1) * RPC, :],
                    rs_b.ap()[k * RPC:(k + 1) * RPC, :])
                add_dep_helper(cp.ins, rs_insts[k].ins, reason="copy rs out")

    nc.compile()
    return nc


def _get_nc(C_use, C_pad, edges, scat_tiles, touch_sets):
    key = (C_use, C_pad, NOLD, NCH,
           tuple(sorted((k, tuple(v)) for k, v in edges.items())),
           tuple(sorted(scat_tiles.items())), touch_sets)
    if key not in _nc_cache:
        _nc_cache[key] = _build(C_use, C_pad, edges, scat_tiles, touch_sets)
    return _nc_cache[key]


def kernel(hidden_states, gate_w, expert_gate, expert_up, expert_down,
           shared_gate, shared_up, shared_down):
    global last_exec_time_ns
    B, S, Hh = hidden_states.shape
    x = np.asarray(hidden_states, np.float32).reshape(-1, Hh)

    # ---- host-side routing: top-k expert choice + combine weights ----
    gw = np.asarray(gate_w, np.float32)
    logits = x @ gw.T
    scores = 1.0 / (1.0 + np.exp(-logits))
    # top-4 per token; stable sort matches jax.lax.top_k tie semantics
    order = np.argsort(-scores, axis=1, kind="stable")[:, :TOPK]
    topk_w = np.take_along_axis(scores, order, axis=1)
    topk_w = topk_w / (topk_w.sum(-1, keepdims=True) + 1e-20)
    w2 = np.zeros((T, E), np.float32)
    np.put_along_axis(w2, order, topk_w, axis=1)
    sel = w2 > 0
    counts = sel.sum(0)
    C_use = int(max(64, -(-int(counts.max()) // 64) * 64))
    C_use = min(C_use, T)
    C_pad = -(-C_use // 128) * 128
    NCC = C_pad // 128

    gidx_all = np.zeros((E, C_pad), np.int32)
    sidx_all = np.full((E, C_pad), OOB, np.int32)
    for e in range(E):
        lst = np.nonzero(sel[:, e])[0].astype(np.int32)
        gidx_all[e, :len(lst)] = lst
        sidx_all[e, :len(lst)] = lst

    # ---- cast / pack per-core inputs (the all-to-all token dispatch),
    # partition-major so each DMA is 128 fat descriptors ----
    x16 = x.astype(np.float16)
    # [4 quarters][128 part (h%128)][8 (h//128)][512 tokens]
    xTq = np.ascontiguousarray(
        x16.T.reshape(H // 128, 128, 4, T // 4).transpose(2, 1, 0, 3))
    eg = np.asarray(expert_gate, np.float32).astype(np.float16)
    eu = np.asarray(expert_up, np.float32).astype(np.float16)
    ed = np.asarray(expert_down, np.float32).astype(np.float16)
    sg = np.asarray(shared_gate, np.float32).astype(np.float16)
    su = np.asarray(shared_up, np.float32).astype(np.float16)
    sd = np.asarray(shared_down, np.float32).astype(np.float16)
    identity = np.eye(128, dtype=np.float16)

    def pack_w(w):  # [H, I] -> [128, H//128, I]
        return np.ascontiguousarray(w.reshape(-1, 128, w.shape[-1]).transpose(1, 0, 2))

    in_maps = []
    for c in range(N_CORES):
        ex = [EPC * c + k for k in range(EPC)]
        xgT = np.stack([
            np.ascontiguousarray(
                x16[gidx_all[e]].T.reshape(H // 128, 128, C_pad).transpose(1, 0, 2))
            for e in ex
        ])
        sgsu = np.concatenate(
            [sg[:, c * SIC:(c + 1) * SIC], su[:, c * SIC:(c + 1) * SIC]], axis=1)
        # combine weights in gathered layout: [128][e*NCC+cc][local e]
        wgt = np.zeros((128, EPC * NCC, EPC), np.float32)
        for k, e in enumerate(ex):
            wgt[:, k * NCC:(k + 1) * NCC, k] = \
                w2[gidx_all[e].reshape(NCC, 128), e].T
        in_maps.append({
            "xTq": xTq,
            "xgT16": xgT,
            "wg16": np.stack([pack_w(eg[e]) for e in ex]),
            "wu16": np.stack([pack_w(eu[e]) for e in ex]),
            "wd16": np.stack([pack_w(ed[e]) for e in ex]),
            "sgsu16": pack_w(sgsu),
            "sd16": np.ascontiguousarray(sd[c * SIC:(c + 1) * SIC, :]),
            "sidx": np.ascontiguousarray(
                sidx_all[ex].reshape(EPC * NCC, 128).T),
            "wgt": wgt,
            "ident": identity,
        })

    # scatter collision edges (union across cores so the SPMD program is
    # identical everywhere), dense-init tiles each scatter touches, and
    # which scatters touch each RS token chunk
    CH = T // NCH
    edge_set = set()
    sctiles = {(k, i): set() for k in range(EPC) for i in range(NCC)}
    touch = [set() for _ in range(NCH)]
    rng = {}
    for c in range(N_CORES):
        for k, e in enumerate([EPC * c, EPC * c + 1]):
            for i in range(NCC):
                r = sidx_all[e, i * 128:(i + 1) * 128]
                r = r[r < OOB]
                if len(r):
                    lo, hi = int(r.min()), int(r.max())
                    rng[(c, k, i)] = (lo, hi)
                    for t in range(lo // 128, hi // 128 + 1):
                        sctiles[(k, i)].add(t)
                    for h in range(lo // CH, hi // CH + 1):
                        touch[h].add((k, i))
        for i in range(NCC):
            for j in range(NCC):
                a = rng.get((c, 0, i))
                b = rng.get((c, 1, j))
                if a and b and a[0] <= b[1] and b[0] <= a[1]:
                    # emission order: (0,cc), (1,cc) per cc ascending
                    if i <= j:
                        edge_set.add(((1, j), (0, i)))
                    else:
                        edge_set.add(((0, i), (1, j)))
    edges = {}
    for later, earlier in edge_set:
        edges.setdefault(later, []).append(earlier)
    edges = {k: tuple(sorted(v)) for k, v in edges.items()}
    scat_tiles = {key: tuple(sorted(v)) for key, v in sctiles.items()}
    touch_sets = tuple(frozenset(s) for s in touch)

    nc = _get_nc(C_use, C_pad, edges, scat_tiles, touch_sets)
    trace = bool(int(os.environ.get("KERNEL_TRACE", "0")))
    res = run_bass_kernel_spmd(
        nc, in_maps, core_ids=list(range(N_CORES)), trace=trace
    )
    last_exec_time_ns = res.exec_time_ns

    # reassemble: RS chunk k gives core c tokens [k*CH + c*RPC, +RPC)
    RPC = T // N_CORES // NCH
    out = np.empty((T, Hh), np.float32)
    for c in range(N_CORES):
        yo = res.results[c]["y_out"]
        for k in range(NCH):
            out[k * CH + c * RPC:k * CH + (c + 1) * RPC] = yo[k * RPC:(k + 1) * RPC]
    return out.reshape(B, S, Hh).astype(np.float32)


# revision 35
# speedup vs baseline: 1.1308x; 1.0448x over previous
"""DeepseekV3 MoE layer on 8 Trainium2 NeuronCores.

Strategy (expert-parallel, per sharding hint):
- Each core owns 2 of the 16 routed experts. The host routes tokens to cores
  by top-k index lists (the all-to-all dispatch, done as input sharding): each
  core receives its experts' gathered tokens pre-transposed to [H, C] fp16,
  plus the per-token combine weights for its experts (host-side routing).
- The device runs the SwiGLU expert MLP in fp16 (fp32 PSUM accumulation),
  scales expert outputs by the combine weights, and scatter-adds them into a
  partial-output buffer in DRAM.
- The shared expert is sharded along its intermediate dim (128 of 1024 per
  core); its partial output initializes the partial-output buffer.
- One ReduceScatter sums the partials across cores; its result is copied to
  the output. The host reassembles the 8 slices and casts to fp32.

All inputs are host-packed partition-major so every DMA moves ~128 fat
descriptors (the DMA rings drain at descriptor-count parity, not bytes).
"""

import os
import sys
import types

sys.path.insert(0, "/opt/trn_rl_repo")

# antenv.axon_hooks shim so trace=True works under axon (profiling only).
if "antenv.axon_hooks" not in sys.modules:
    _hook_holder = [None]
    _hooks_mod = types.ModuleType("antenv.axon_hooks")
    _hooks_mod.set_axon_ntff_profile_hook = lambda h: _hook_holder.__setitem__(0, h)
    _hooks_mod.get_axon_ntff_profile_hook = lambda: _hook_holder[0]
    sys.modules["antenv.axon_hooks"] = _hooks_mod
    try:
        from trn_agent_boot.trn_boot import _ntff_profile_via_ctypes

        _hook_holder[0] = _ntff_profile_via_ctypes("/opt/axon/libaxon_pjrt.so")
    except Exception:
        pass

import numpy as np

import concourse.bass as bass
import concourse.mybir as mybir
from concourse import bacc
from concourse.tile import TileContext, add_dep_helper
from concourse.bass_utils import run_bass_kernel_spmd

N_CORES = 8
T, H, E, I = 2048, 1024, 16, 512
TOPK = 4
SIC = 128  # shared-expert intermediate slice per core (1024 / 8)
EPC = 2  # experts per core
OOB = 1 << 20
NOLD = bool(int(os.environ.get("KERNEL_NOLD", "1")))  # skip dup weight loads
NCH = int(os.environ.get("KERNEL_NCH", "1"))  # reduce-scatter token chunks

F16 = mybir.dt.float16
F32 = mybir.dt.float32
I32 = mybir.dt.int32
AF = mybir.ActivationFunctionType

_nc_cache = {}
last_exec_time_ns = None


def _build(C_use, C_pad, edges, scat_tiles, touch_sets):
    NCC = C_pad // 128
    NTI = T // 128
    SS = 2 * SIC  # 256
    ROWS = T // N_CORES  # 256 output rows per core
    nc = bacc.Bacc(trn_type="TRN2", target_bir_lowering=False, num_devices=N_CORES)

    # ---- I/O (all packed partition-major on the host) ----
    xTq = nc.dram_tensor("xTq", [4, 128, H // 128, T // 4], F16, kind="ExternalInput")
    xgT16 = nc.dram_tensor("xgT16", [EPC, 128, H // 128, C_pad], F16, kind="ExternalInput")
    wg16 = nc.dram_tensor("wg16", [EPC, 128, H // 128, I], F16, kind="ExternalInput")
    wu16 = nc.dram_tensor("wu16", [EPC, 128, H // 128, I], F16, kind="ExternalInput")
    wd16 = nc.dram_tensor("wd16", [EPC, 128, I // 128, H], F16, kind="ExternalInput")
    sgsu16 = nc.dram_tensor("sgsu16", [128, H // 128, SS], F16, kind="ExternalInput")
    sd16 = nc.dram_tensor("sd16", [SIC, H], F16, kind="ExternalInput")
    sidx = nc.dram_tensor("sidx", [128, EPC * NCC], I32, kind="ExternalInput")
    wgt = nc.dram_tensor("wgt", [128, EPC * NCC, EPC], F32, kind="ExternalInput")
    ident = nc.dram_tensor("ident", [128, 128], F16, kind="ExternalInput")

    y_acc = nc.dram_tensor("y_acc", [T, H], F16)
    rs_b = nc.dram_tensor("rs_b", [ROWS, H], F16)
    y_out = nc.dram_tensor("y_out", [ROWS, H], F16, kind="ExternalOutput")

    with TileContext(nc) as tc:
        with (
            tc.tile_pool(name="res", bufs=1) as res,
            tc.tile_pool(name="sc", bufs=3) as scp,
            tc.tile_pool(name="yg", bufs=4) as ygp,
            tc.tile_pool(name="ps", bufs=2, space="PSUM") as ps,
        ):
            # ---- resident tiles ----
            # quarter-major so each xT quarter lands as one 8KB run/partition
            xT_sb = res.tile([128, 4, H // 128, T // 4], F16, tag="xT")
            xgT_sb = res.tile([128, EPC, H // 128, C_pad], F16, tag="xgT")
            wg_sb = res.tile([128, EPC, H // 128, I], F16, tag="wg")
            wu_sb = res.tile([128, EPC, H // 128, I], F16, tag="wu")
            wd_sb = res.tile([128, EPC, I // 128, H], F16, tag="wd")
            sgsu_sb = res.tile([128, H // 128, SS], F16, tag="sgsu")
            sd_sb = res.tile([128, H], F16, tag="sd")
            sidx_sb = res.tile([128, EPC * NCC], I32, tag="sidx")
            wG_sb = res.tile([128, EPC * NCC, EPC], F32, tag="wG")
            id_sb = res.tile([128, 128], F16, tag="ident")
            p_sb = res.tile([128, EPC, I // 128, C_pad], F16, tag="p")
            spT_sb = res.tile([128, T], F16, tag="spT")

            # ---- preload (sgsu + xT q0 gate the first matmuls) ----
            sgsu_wr = nc.scalar.dma_start(sgsu_sb[:], sgsu16.ap())
            nc.scalar.dma_start(id_sb[:], ident[:])
            nc.scalar.dma_start(sd_sb[:], sd16.ap())
            nc.scalar.dma_start(sidx_sb[:], sidx.ap())
            nc.scalar.dma_start(wG_sb[:], wgt.ap())

            TC = T // 4
            xt_wr = []
            for q in range(4):
                xt_wr.append(nc.sync.dma_start(xT_sb[:, q], xTq.ap()[q]))
            # gate all bulk traffic on the two loads that unblock compute
            # (the tile scheduler reorders DMAs, so every one needs the dep).
            # weights + xgT all ride the gpsimd ring: the sync ring must be
            # free for the dense y_acc writes right after the xT quarters,
            # since those gate the first scatters.
            bulk = []
            for e in range(EPC):
                bulk.append(nc.gpsimd.dma_start(wg_sb[:, e], wg16.ap()[e]))
                bulk.append(nc.gpsimd.dma_start(wu_sb[:, e], wu16.ap()[e]))
                bulk.append(nc.gpsimd.dma_start(xgT_sb[:, e], xgT16.ap()[e]))
            for e in range(EPC):
                bulk.append(nc.gpsimd.dma_start(wd_sb[:, e], wd16.ap()[e]))
            for b in bulk:
                add_dep_helper(b.ins, xt_wr[0].ins, reason="priority: xT q0 first")
                add_dep_helper(b.ins, sgsu_wr.ins, reason="priority: sgsu first")



            # ---- shared expert, per xT quarter, weight-stationary so the
            # intermediate comes out already transposed (no PE transposes):
            # sgT/suT = (x @ sg|su)^T accumulate [128 si, 512 tok] ----
            dense_wr = []
            for q in range(4):
                sgp = ps.tile([128, TC], F32, tag="A")
                for ho in range(H // 128):
                    nc.tensor.matmul(
                        sgp[:],
                        lhsT=sgsu_sb[:, ho, 0:SIC],
                        rhs=xT_sb[:, q, ho, :],
                        start=(ho == 0),
                        stop=(ho == H // 128 - 1),
                    )
                sgq = scp.tile([128, TC], F16, tag="sgq")
                nc.scalar.activation(sgq[:], sgp[:], AF.Silu)
                sup = ps.tile([128, TC], F32, tag="B")
                for ho in range(H // 128):
                    nc.tensor.matmul(
                        sup[:],
                        lhsT=sgsu_sb[:, ho, SIC:SS],
                        rhs=xT_sb[:, q, ho, :],
                        start=(ho == 0),
                        stop=(ho == H // 128 - 1),
                    )
                nc.vector.tensor_tensor(
                    out=spT_sb[:, q * TC:(q + 1) * TC], in0=sgq[:], in1=sup[:],
                    op=mybir.AluOpType.mult,
                )
                for ti in range(4 * q, 4 * q + 4):
                    ysh = ps.tile([128, H], F32, tag="Y")
                    for hf in range(2):
                        nc.tensor.matmul(
                            ysh[:, hf * 512:(hf + 1) * 512],
                            lhsT=spT_sb[:, ti * 128:(ti + 1) * 128],
                            rhs=sd_sb[:, hf * 512:(hf + 1) * 512],
                            start=True,
                            stop=True,
                        )
                    ys = ygp.tile([128, H], F16, tag="ys", bufs=4)
                    nc.scalar.activation(ys[:], ysh[:], AF.Copy)
                    # alternate rings: each drains 2KB-descriptor writes at
                    # ~87GB/s, so two rings halve the dense-init drain
                    qr = nc.sync if ti % 2 == 0 else nc.scalar
                    wr = qr.dma_start(out=y_acc[ti * 128:(ti + 1) * 128, :], in_=ys[:])
                    dense_wr.append(wr)

            # ---- routed experts: per 128-token chunk, fully fused:
            # gate/up (token-stationary: one xgT tile load feeds both 512-wide
            # streams), silu*up, transpose back to I-major, down, scale,
            # scatter. Scatters start ~45us in and overlap all later compute.
            scat_insts = {}

            def emit_gu_cc(e, cc):
                a = cc * 128
                pgT = ps.tile([128, I], F32, tag="A")
                puT = ps.tile([128, I], F32, tag="B")
                for ho in range(H // 128):
                    nc.tensor.matmul(
                        pgT[:],
                        lhsT=xgT_sb[:, e, ho, a:a + 128],
                        rhs=wg_sb[:, e, ho, :],
                        start=(ho == 0),
                        stop=(ho == H // 128 - 1),
                    )
                    nc.tensor.matmul(
                        puT[:],
                        lhsT=xgT_sb[:, e, ho, a:a + 128],
                        rhs=wu_sb[:, e, ho, :],
                        start=(ho == 0),
                        stop=(ho == H // 128 - 1),
                    )
                sg2 = scp.tile([128, I], F16, tag="sg2")
                nc.scalar.activation(sg2[:], pgT[:], AF.Silu)
                sp2 = scp.tile([128, I], F16, tag="sp2")
                nc.vector.tensor_tensor(
                    out=sp2[:], in0=sg2[:], in1=puT[:],
                    op=mybir.AluOpType.mult,
                )
                for it in range(I // 128):
                    tps2 = ps.tile([128, 128], F16, tag="B")
                    nc.tensor.transpose(
                        tps2[:], sp2[:, it * 128:(it + 1) * 128], id_sb[:])
                    nc.vector.tensor_copy(p_sb[:, e, it, a:a + 128], tps2[:])

            def emit_down(e, cc):
                j = e * NCC + cc
                py = ps.tile([128, H], F32, tag="Y")
                for it in range(I // 128):
                    for hf in range(2):
                        mm = nc.tensor.matmul(
                            py[:, hf * 512:(hf + 1) * 512],
                            lhsT=p_sb[:, e, it, cc * 128:(cc + 1) * 128],
                            rhs=wd_sb[:, e, it, hf * 512:(hf + 1) * 512],
                            start=(it == 0),
                            stop=(it == I // 128 - 1),
                        )
                        if NOLD and hf == 1:
                            mm.ins.ldweights = False
                yg = ygp.tile([128, H], F16, tag="ygtile", bufs=6)
                nc.vector.tensor_scalar_mul(yg[:], py[:], wG_sb[:, j, e:e + 1])

                sc = nc.gpsimd.indirect_dma_start(
                    out=y_acc[:],
                    out_offset=bass.IndirectOffsetOnAxis(
                        ap=sidx_sb[:, j:j + 1], axis=0),
                    in_=yg[:],
                    in_offset=None,
                    bounds_check=T - 1,
                    oob_is_err=False,
                    compute_op=mybir.AluOpType.add,
                )
                # order RMW scatters after the dense init of the tiles they
                # touch, and serialize only colliding scatter pairs
                for t in scat_tiles[(e, cc)]:
                    add_dep_helper(sc.ins, dense_wr[t].ins, reason="scatter after dense init")
                for (oe, occ) in edges.get((e, cc), ()):
                    if (oe, occ) in scat_insts:
                        add_dep_helper(sc.ins, scat_insts[(oe, occ)].ins,
                                       reason="serialize colliding scatters")
                scat_insts[(e, cc)] = sc

            # ---- token-chunked ReduceScatter, triggered progressively ----
            rs_insts = []
            CH = T // NCH
            RPC = ROWS // NCH

            def emit_rs(k):
                cc_inst = nc.gpsimd.collective_compute(
                    "ReduceScatter",
                    mybir.AluOpType.add,
                    replica_groups=[list(range(N_CORES))],
                    ins=[y_acc.ap()[k * CH:(k + 1) * CH, :].opt()],
                    outs=[rs_b.ap()[k * RPC:(k + 1) * RPC, :].opt()],
                )
                for key in touch_sets[k]:
                    add_dep_helper(cc_inst.ins, scat_insts[key].ins,
                                   reason="rs after scatters")
                for t in range(k * (NTI // NCH), (k + 1) * (NTI // NCH)):
                    add_dep_helper(cc_inst.ins, dense_wr[t].ins,
                                   reason="rs after dense init")
                rs_insts.append(cc_inst)

            def try_emit_rs():
                while len(rs_insts) < NCH and all(
                        key in scat_insts for key in touch_sets[len(rs_insts)]):
                    emit_rs(len(rs_insts))

            for cc in range(NCC):
                for e in range(EPC):
                    emit_gu_cc(e, cc)
                    emit_down(e, cc)
                try_emit_rs()
            try_emit_rs()
            assert len(rs_insts) == NCH, f"only {len(rs_insts)} RS chunks emitted"

            for k in range(NCH):
                cp = nc.sync.dma_start(
                    y_out.ap()[k * RPC:(k + 1) * RPC, :],
                    rs_b.ap()[k * RPC:(k + 1) * RPC, :])
                add_dep_helper(cp.ins, rs_insts[k].ins, reason="copy rs out")

    nc.compile()
    return nc


def _get_nc(C_use, C_pad, edges, scat_tiles, touch_sets):
    key = (C_use, C_pad, NOLD, NCH,
           tuple(sorted((k, tuple(v)) for k, v in edges.items())),
           tuple(sorted(scat_tiles.items())), touch_sets)
    if key not in _nc_cache:
        _nc_cache[key] = _build(C_use, C_pad, edges, scat_tiles, touch_sets)
    return _nc_cache[key]


def kernel(hidden_states, gate_w, expert_gate, expert_up, expert_down,
           shared_gate, shared_up, shared_down):
    global last_exec_time_ns
    B, S, Hh = hidden_states.shape
    x = np.asarray(hidden_states, np.float32).reshape(-1, Hh)

    # ---- host-side routing: top-k expert choice + combine weights ----
    gw = np.asarray(gate_w, np.float32)
    logits = x @ gw.T
    scores = 1.0 / (1.0 + np.exp(-logits))
    # top-4 per token; stable sort matches jax.lax.top_k tie semantics
    order = np.argsort(-scores, axis=1, kind="stable")[:, :TOPK]
    topk_w = np.take_along_axis(scores, order, axis=1)
    topk_w = topk_w / (topk_w.sum(-1, keepdims=True) + 1e-20)
    w2 = np.zeros((T, E), np.float32)
    np.put_along_axis(w2, order, topk_w, axis=1)
    sel = w2 > 0
    counts = sel.sum(0)
    C_use = int(max(64, -(-int(counts.max()) // 64) * 64))
    C_use = min(C_use, T)
    C_pad = -(-C_use // 128) * 128
    NCC = C_pad // 128

    gidx_all = np.zeros((E, C_pad), np.int32)
    sidx_all = np.full((E, C_pad), OOB, np.int32)
    for e in range(E):
        lst = np.nonzero(sel[:, e])[0].astype(np.int32)
        gidx_all[e, :len(lst)] = lst
        sidx_all[e, :len(lst)] = lst

    # ---- cast / pack per-core inputs (the all-to-all token dispatch),
    # partition-major so each DMA is 128 fat descriptors ----
    x16 = x.astype(np.float16)
    # [4 quarters][128 part (h%128)][8 (h//128)][512 tokens]
    xTq = np.ascontiguousarray(
        x16.T.reshape(H // 128, 128, 4, T // 4).transpose(2, 1, 0, 3))
    eg = np.asarray(expert_gate, np.float32).astype(np.float16)
    eu = np.asarray(expert_up, np.float32).astype(np.float16)
    ed = np.asarray(expert_down, np.float32).astype(np.float16)
    sg = np.asarray(shared_gate, np.float32).astype(np.float16)
    su = np.asarray(shared_up, np.float32).astype(np.float16)
    sd = np.asarray(shared_down, np.float32).astype(np.float16)
    identity = np.eye(128, dtype=np.float16)

    def pack_w(w):  # [H, I] -> [128, H//128, I]
        return np.ascontiguousarray(w.reshape(-1, 128, w.shape[-1]).transpose(1, 0, 2))

    in_maps = []
    for c in range(N_CORES):
        ex = [EPC * c + k for k in range(EPC)]
        xgT = np.stack([
            np.ascontiguousarray(
                x16[gidx_all[e]].T.reshape(H // 128, 128, C_pad).transpose(1, 0, 2))
            for e in ex
        ])
        sgsu = np.concatenate(
            [sg[:, c * SIC:(c + 1) * SIC], su[:, c * SIC:(c + 1) * SIC]], axis=1)
        # combine weights in gathered layout: [128][e*NCC+cc][local e]
        wgt = np.zeros((128, EPC * NCC, EPC), np.float32)
        for k, e in enumerate(ex):
            wgt[:, k * NCC:(k + 1) * NCC, k] = \
                w2[gidx_all[e].reshape(NCC, 128), e].T
        in_maps.append({
            "xTq": xTq,
            "xgT16": xgT,
            "wg16": np.stack([pack_w(eg[e]) for e in ex]),
            "wu16": np.stack([pack_w(eu[e]) for e in ex]),
            "wd16": np.stack([pack_w(ed[e]) for e in ex]),
            "sgsu16": pack_w(sgsu),
            "sd16": np.ascontiguousarray(sd[c * SIC:(c + 1) * SIC, :]),
            "sidx": np.ascontiguousarray(
                sidx_all[ex].reshape(EPC * NCC, 128).T),
            "wgt": wgt,
            "ident": identity,
        })

    # scatter collision edges (union across cores so the SPMD program is
    # identical everywhere), dense-init tiles each scatter touches, and
    # which scatters touch each RS token chunk
    CH = T // NCH
    edge_set = set()
    sctiles = {(k, i): set() for k in range(EPC) for i in range(NCC)}
    touch = [set() for _ in range(NCH)]
    rng = {}
    for c in range(N_CORES):
        for k, e in enumerate([EPC * c, EPC * c + 1]):
            for i in range(NCC):
                r = sidx_all[e, i * 128:(i + 1) * 128]
                r = r[r < OOB]
                if len(r):
                    lo, hi = int(r.min()), int(r.max())
                    rng[(c, k, i)] = (lo, hi)
                    for t in range(lo // 128, hi // 128 + 1):
                        sctiles[(k, i)].add(t)
                    for h in range(lo // CH, hi // CH + 1):
                        touch[h].add((k, i))
        for i in range(NCC):
            for j in range(NCC):
                a = rng.get((c, 0, i))
                b = rng.get((c, 1, j))
                if a and b and a[0] <= b[1] and b[0] <= a[1]:
                    # emission order: (0,cc), (1,cc) per cc ascending
                    if i <= j:
                        edge_set.add(((1, j), (0, i)))
                    else:
                        edge_set.add(((0, i), (1, j)))
    edges = {}
    for later, earlier in edge_set:
        edges.setdefault(later, []).append(earlier)
    edges = {k: tuple(sorted(v)) for k, v in edges.items()}
    scat_tiles = {key: tuple(sorted(v)) for key, v in sctiles.items()}
    touch_sets = tuple(frozenset(s) for s in touch)

    nc = _get_nc(C_use, C_pad, edges, scat_tiles, touch_sets)
    trace = bool(int(os.environ.get("KERNEL_TRACE", "0")))
    res = run_bass_kernel_spmd(
        nc, in_maps, core_ids=list(range(N_CORES)), trace=trace
    )
    last_exec_time_ns = res.exec_time_ns

    # reassemble: RS chunk k gives core c tokens [k*CH + c*RPC, +RPC)
    RPC = T // N_CORES // NCH
    out = np.empty((T, Hh), np.float32)
    for c in range(N_CORES):
        yo = res.results[c]["y_out"]
        for k in range(NCH):
            out[k * CH + c * RPC:k * CH + (c + 1) * RPC] = yo[k * RPC:(k + 1) * RPC]
    return out.reshape(B, S, Hh).astype(np.float32)
